# revision 1
# baseline (speedup 1.0000x reference)
"""Trainium2 Bass kernel for nn_EqStftPBC (STFT perturbation-based compensation).

Per (batch b, mode m):
  X = STFT(x); C_n2 = X*conj(roll(X,n2)) + prev-frame; U_n2 = circulant(w[:,n2]);
  V_n2 = U_n2 * roll(X,n2); delta = sum_n2 V_n2; out = x + ISTFT(delta)*P (+bias)

8 cores = (b x m x n2-half), uniform SPMD program; per-core variation only in
input data (permutation stack S, circulant stack M).  Device layout: [freq(80)
partitions, time free].  STFT fp32, rest bf16 (PSUM fp32).
"""

import numpy as np
from ml_dtypes import bfloat16

import concourse.bass as bass
import concourse.bacc as bacc
import concourse.mybir as mybir
import concourse.tile as tile

F = 80
T = 51
TP = 52          # padded slot stride
HOP = 40
L = 2080
NJ = 20
NCH = 2
CHJ = NJ // NCH
PBK = 5          # stage-1/R psum outputs per bank
GJ = 5           # j per merged G-matmul (N = GJ*102 <= 512)
FP32 = mybir.dt.float32
BF16 = mybir.dt.bfloat16

N2_LISTS = [list(range(19, -1, -1)), list(range(-1, -21, -1))]


def _dft_consts():
    j = np.arange(F)
    W = np.exp(-2j * np.pi * np.outer(j, j) / F)
    G = np.exp(+2j * np.pi * np.outer(j, j) / F) / F
    return W, G


def build_program(debug=False):
    nc = bacc.Bacc("TRN2", target_bir_lowering=False, debug=debug)

    # xf = [fiN | fr | fi] frames, pre-framed on host (pure reshape)
    xf = nc.dram_tensor("xf", [F, 3 * T], BF16, kind="ExternalInput")
    fr_c = nc.dram_tensor("fr_c", [F, 2 * F], BF16, kind="ExternalInput")
    gr_c = nc.dram_tensor("gr_c", [F, 2 * F], BF16, kind="ExternalInput")
    smat = nc.dram_tensor("smat", [F, NJ * F], BF16, kind="ExternalInput")
    mst = nc.dram_tensor("mst", [F, NJ * 2 * F], BF16, kind="ExternalInput")
    svec = nc.dram_tensor("svec", [HOP, 52], FP32, kind="ExternalInput")
    yv = nc.dram_tensor("yv", [HOP, 2 * 52], FP32, kind="ExternalOutput")

    with tile.TileContext(nc) as tc:
        with (
            tc.tile_pool(name="const", bufs=1) as cpool,
            tc.tile_pool(name="work", bufs=1) as wpool,
            tc.tile_pool(name="ps_x", bufs=1, space="PSUM") as ps_x,
            tc.tile_pool(name="ps_r", bufs=2, space="PSUM") as ps_r,
            tc.tile_pool(name="ps_u", bufs=2, space="PSUM") as ps_u,
            tc.tile_pool(name="ps_d", bufs=1, space="PSUM") as ps_d,
        ):
            frm = wpool.tile([F, 3 * T], BF16, tag="frm")
            nc.sync.dma_start(frm[:, :], xf[:, :])
            Fc = cpool.tile([F, 2 * F], BF16, tag="Fc")
            nc.sync.dma_start(Fc[:, :], fr_c[:, :])
            Ssb = cpool.tile([F, NJ * F], BF16, tag="Ssb")
            for q in range(NJ // PBK):
                nc.sync.dma_start(Ssb[:, q * PBK * F:(q + 1) * PBK * F],
                                  smat[:, q * PBK * F:(q + 1) * PBK * F])
            Msb = cpool.tile([F, NJ * 2 * F], BF16, tag="Msb")
            for c in range(NCH):
                nc.gpsimd.dma_start(Msb[:, c * CHJ * 2 * F:(c + 1) * CHJ * 2 * F],
                                    mst[:, c * CHJ * 2 * F:(c + 1) * CHJ * 2 * F])
            Gc = cpool.tile([F, 2 * F], BF16, tag="Gc")
            nc.gpsimd.dma_start(Gc[:, :], gr_c[:, :])
            sv = cpool.tile([HOP, 52], FP32, tag="sv")
            nc.gpsimd.dma_start(sv[:, :], svec[:, :])

            # ---- STFT (fp32) -> X bf16 [Xr(52) | Xi(52)] ----
            Xp = ps_x.tile([F, 2 * T], FP32, tag="Xp")
            nc.tensor.matmul(Xp[:, :], Fc[:, 0:F], frm[:, T:3 * T], start=True, stop=False)
            nc.tensor.matmul(Xp[:, :], Fc[:, F:2 * F], frm[:, 0:2 * T], start=False, stop=True)
            Xsb = wpool.tile([F, 2 * TP], BF16, tag="Xsb")
            Xsv = Xsb[:, :].rearrange("p (c t) -> p c t", c=2)
            nc.scalar.activation(Xsv[:, :, 0:T],
                                 Xp[:, :].rearrange("p (c t) -> p c t", c=2),
                                 mybir.ActivationFunctionType.Copy)
            Xrhs = bass.AP(tensor=Xsb[:, :].tensor, offset=Xsb[:, :].offset,
                           ap=[[2 * TP, F], [TP, 2], [1, T]])

            # plane-major per-chunk stacks: R/U = [r-block | i-block], blocks CHJ*TP
            # C/V = [negi-block | r-block | i-block]
            BL = CHJ * TP
            Rsb, Csb, Usb, Vsb = [], [], [], []
            for c in range(NCH):
                Rsb.append(wpool.tile([F, 2 * BL], BF16, tag=f"Rsb{c}", name=f"Rsb{c}"))
                Csb.append(wpool.tile([F, 3 * BL], BF16, tag=f"Csb{c}", name=f"Csb{c}"))
                Usb.append(wpool.tile([F, 2 * BL], BF16, tag=f"Usb{c}", name=f"Usb{c}"))
                Vsb.append(wpool.tile([F, 3 * BL], BF16, tag=f"Vsb{c}", name=f"Vsb{c}"))
            sA = wpool.tile([F, BL], BF16, tag="sA")
            sB = wpool.tile([F, BL], BF16, tag="sB")
            sC = wpool.tile([F, BL], BF16, tag="sC")
            sD = wpool.tile([F, BL], BF16, tag="sD")
            sPR = wpool.tile([F, BL], BF16, tag="sPR")
            sPI = wpool.tile([F, BL], BF16, tag="sPI")

            Dps = [ps_d.tile([F, GJ * 2 * T], FP32, tag=f"Dp{c}", name=f"Dp{c}")
                   for c in range(NCH)]  # per-chunk accumulated [dr|di] pairs

            TT = nc.vector.tensor_tensor
            TG = nc.gpsimd.tensor_tensor
            MUL = mybir.AluOpType.mult
            ADD = mybir.AluOpType.add
            SUB = mybir.AluOpType.subtract
            CPY = mybir.ActivationFunctionType.Copy

            Dcs = []
            for c in range(NCH):
                Rc, Cc, Uc, Vc = Rsb[c], Csb[c], Usb[c], Vsb[c]
                # ---- R: permutation matmuls, PBK per bank, plane-major evict ----
                for bk in range(CHJ // PBK):
                    Rp = ps_r.tile([F, PBK * 2 * T], FP32, tag="Rp")
                    for s in range(PBK):
                        j = c * CHJ + bk * PBK + s
                        nc.tensor.matmul(Rp[:, s * 2 * T:(s + 1) * 2 * T],
                                         Ssb[:, j * F:(j + 1) * F],
                                         Xrhs, start=True, stop=True)
                    # psum [s, c2, t] -> Rsb [c2-block, (bk*PBK+s)*TP + t]
                    dst = bass.AP(tensor=Rc[:, :].tensor,
                                  offset=Rc[:, :].offset + bk * PBK * TP,
                                  ap=[[2 * BL, F], [TP, PBK], [BL, 2], [1, T]])
                    nc.scalar.activation(
                        dst, Rp[:, :].rearrange("p (s c2 t) -> p s c2 t", s=PBK, c2=2),
                        CPY)

                Rrf = Rc[:, 0:BL]
                Rif = Rc[:, BL:2 * BL]
                vPR = sPR[:, :].rearrange("p (j t) -> p j t", j=CHJ)
                vPI = sPI[:, :].rearrange("p (j t) -> p j t", j=CHJ)

                # ---- C_pre = X * conj(R)  (flat 2D ops; Xt = tiled X copies) ----
                if c == 0:
                    Xtr = wpool.tile([F, BL], BF16, tag="Xtr")
                    Xti = wpool.tile([F, BL], BF16, tag="Xti")
                    nc.scalar.activation(
                        Xtr[:, :].rearrange("p (j t) -> p j t", j=CHJ),
                        Xsb[:, None, 0:TP].to_broadcast([F, CHJ, TP]), CPY)
                    nc.scalar.activation(
                        Xti[:, :].rearrange("p (j t) -> p j t", j=CHJ),
                        Xsb[:, None, TP:2 * TP].to_broadcast([F, CHJ, TP]), CPY)
                TT(sA[:, :], Xtr[:, :], Rrf, MUL)
                TT(sB[:, :], Xti[:, :], Rif, MUL)
                TT(sPR[:, :], sA[:, :], sB[:, :], ADD)
                TG(sC[:, :], Xti[:, :], Rrf, MUL)
                TG(sD[:, :], Xtr[:, :], Rif, MUL)
                TG(sPI[:, :], sC[:, :], sD[:, :], SUB)

                # ---- C = C_pre + roll_t;  blocks [CiN | Cr | Ci] ----
                CrB = Cc[:, BL:2 * BL].rearrange("p (j t) -> p j t", j=CHJ)
                CiB = Cc[:, 2 * BL:3 * BL].rearrange("p (j t) -> p j t", j=CHJ)
                TT(CrB[:, :, 1:T], vPR[:, :, 1:T], vPR[:, :, 0:T - 1], ADD)
                TT(CrB[:, :, 0:1], vPR[:, :, 0:1], vPR[:, :, T - 1:T], ADD)
                TG(CiB[:, :, 1:T], vPI[:, :, 1:T], vPI[:, :, 0:T - 1], ADD)
                TG(CiB[:, :, 0:1], vPI[:, :, 0:1], vPI[:, :, T - 1:T], ADD)
                nc.scalar.activation(Cc[:, 0:BL], Cc[:, 2 * BL:3 * BL], CPY, scale=-1.0)

                # ---- stage-1: U_j = Mr.T@[Cr|Ci] + Mi.T@[CiN|Cr] ----
                for bk in range(CHJ // PBK):
                    Up = ps_u.tile([F, PBK * 2 * T], FP32, tag="Up")
                    for s in range(PBK):
                        jj = bk * PBK + s
                        j = c * CHJ + jj
                        rhs1 = bass.AP(tensor=Cc[:, :].tensor,
                                       offset=Cc[:, :].offset + BL + jj * TP,
                                       ap=[[3 * BL, F], [BL, 2], [1, T]])
                        rhs2 = bass.AP(tensor=Cc[:, :].tensor,
                                       offset=Cc[:, :].offset + jj * TP,
                                       ap=[[3 * BL, F], [BL, 2], [1, T]])
                        nc.tensor.matmul(Up[:, s * 2 * T:(s + 1) * 2 * T],
                                         Msb[:, (2 * j) * F:(2 * j + 1) * F],
                                         rhs1, start=True, stop=False)
                        nc.tensor.matmul(Up[:, s * 2 * T:(s + 1) * 2 * T],
                                         Msb[:, (2 * j + 1) * F:(2 * j + 2) * F],
                                         rhs2, start=False, stop=True)
                    dst = bass.AP(tensor=Uc[:, :].tensor,
                                  offset=Uc[:, :].offset + bk * PBK * TP,
                                  ap=[[2 * BL, F], [TP, PBK], [BL, 2], [1, T]])
                    nc.scalar.activation(
                        dst, Up[:, :].rearrange("p (s c2 t) -> p s c2 t", s=PBK, c2=2),
                        CPY)

                # ---- stage-2: V = U * R;  blocks [ViN | Vr | Vi]  (flat 2D) ----
                Urf = Uc[:, 0:BL]
                Uif = Uc[:, BL:2 * BL]
                TT(sA[:, :], Urf, Rrf, MUL)
                TT(sB[:, :], Uif, Rif, MUL)
                TT(Vc[:, BL:2 * BL], sA[:, :], sB[:, :], SUB)
                TG(sC[:, :], Urf, Rif, MUL)
                TG(sD[:, :], Uif, Rrf, MUL)
                TG(Vc[:, 2 * BL:3 * BL], sC[:, :], sD[:, :], ADD)
                nc.scalar.activation(Vc[:, 0:BL], Vc[:, 2 * BL:3 * BL], CPY, scale=-1.0)

                # ---- merged G-matmuls: accumulate into 5 [dr|di] pairs ----
                for gpass in range(2):
                    for h in range(CHJ // GJ):
                        base = (BL if gpass == 0 else 0) + h * GJ * TP
                        rhs = bass.AP(tensor=Vc[:, :].tensor,
                                      offset=Vc[:, :].offset + base,
                                      ap=[[3 * BL, F], [TP, GJ], [BL, 2], [1, T]])
                        nc.tensor.matmul(
                            Dps[c][:, :].rearrange("p (s c2 t) -> p s c2 t", s=GJ, c2=2),
                            Gc[:, gpass * F:(gpass + 1) * F], rhs,
                            start=(gpass == 0 and h == 0),
                            stop=(gpass == 1 and h == CHJ // GJ - 1))

                # per-chunk partial reduce (overlaps next chunk): 5 pairs -> 1
                D5 = wpool.tile([F, GJ * 2 * T], FP32, tag=f"D5{c}", name=f"D5{c}")
                nc.scalar.activation(D5[:, :], Dps[c][:, :], CPY)
                tE = wpool.tile([F, 4 * T], FP32, tag=f"tE{c}", name=f"tE{c}")
                TT(tE[:, :], D5[:, 0:4 * T], D5[:, 4 * T:8 * T], ADD)
                tF = wpool.tile([F, 2 * T], FP32, tag=f"tF{c}", name=f"tF{c}")
                TT(tF[:, :], tE[:, 0:2 * T], tE[:, 2 * T:4 * T], ADD)
                Dcs.append((tF, D5))

            # ---- final cross-chunk reduce + overlap-add + scale (fp32) ----
            tG = wpool.tile([F, 2 * T], FP32, tag="tG")
            TT(tG[:, :], Dcs[0][0][:, :], Dcs[1][0][:, :], ADD)
            tH = wpool.tile([F, 2 * T], FP32, tag="tH")
            TT(tH[:, :], Dcs[0][1][:, 8 * T:10 * T], Dcs[1][1][:, 8 * T:10 * T], ADD)
            Dsb = wpool.tile([F, 2 * T], FP32, tag="Dsb")
            TT(Dsb[:, :], tG[:, :], tH[:, :], ADD)

            S2 = wpool.tile([HOP, 2 * T], FP32, tag="S2")
            nc.sync.dma_start(S2[:, :], Dsb[HOP:F, :])
            Y = wpool.tile([HOP, 2 * 52], FP32, tag="Y")
            S1v = Dsb[0:HOP, :].rearrange("p (c t) -> p c t", c=2)
            S2v = S2[:, :].rearrange("p (c t) -> p c t", c=2)
            Yv = Y[:, :].rearrange("p (c t) -> p c t", c=2)
            TT(Yv[:, :, 1:T], S1v[:, :, 1:T], S2v[:, :, 0:T - 1], ADD)
            nc.scalar.activation(Yv[:, :, 0:1], S1v[:, :, 0:1], CPY)
            nc.scalar.activation(Yv[:, :, T:52], S2v[:, :, T - 1:T], CPY)
            TT(Yv, Yv, sv[:, None, :].to_broadcast([HOP, 2, 52]), MUL)
            nc.sync.dma_start(yv[:, :], Y[:, :])
    return nc


# ---------------- host side ----------------

def _host_consts():
    W, G = _dft_consts()
    fr_c = np.concatenate([W.real, W.imag], axis=1).astype(bfloat16)
    gr_c = np.concatenate([G.real, G.imag], axis=1).astype(bfloat16)
    cov = np.zeros(L)
    idx = (np.arange(T)[:, None] * HOP + np.arange(F)[None, :]).reshape(-1)
    np.add.at(cov, idx, 1.0)
    cov = np.where(cov > 0, cov, 1.0)
    return fr_c, gr_c, cov


def _smat_for(n2_list):
    S = np.zeros((NJ, F, F), np.float32)
    g = np.arange(F)
    for j, n2 in enumerate(n2_list):
        S[j, (g - n2) % F, g] = 1.0
    return np.ascontiguousarray(S.transpose(1, 0, 2).reshape(F, NJ * F)).astype(bfloat16)


def _mst_for(n2_list, w2):
    Ms = np.zeros((NJ, 2, F, F), np.float32)
    g = np.arange(F)[:, None]
    f = np.arange(F)[None, :]
    n1 = ((f - g + 20) % F) - 20
    valid = (n1 >= -20) & (n1 <= 19)
    n1c = np.clip(n1 + 20, 0, 39)
    for j, n2 in enumerate(n2_list):
        col = w2[:, n2 + 20]
        Ms[j, 0] = np.where(valid, col.real[n1c], 0.0)
        Ms[j, 1] = np.where(valid, col.imag[n1c], 0.0)
    return np.ascontiguousarray(
        Ms.transpose(2, 0, 1, 3).reshape(F, NJ * 2 * F)).astype(bfloat16)


def _frame(sig):
    idx = np.arange(T)[None, :] * HOP + np.arange(F)[:, None]   # [j, t]
    return sig[idx].astype(np.float32)


def make_in_maps(x_real, x_imag, task_info, w_real, w_imag):
    fr_c, gr_c, cov = _host_consts()
    b, _, m = x_real.shape
    P = np.power(10.0, task_info[:, 0] / 10.0) / m
    w2 = (np.asarray(w_real) + 1j * np.asarray(w_imag)).reshape(40, 40)
    smats = [_smat_for(nl) for nl in N2_LISTS]
    msts = [_mst_for(nl, w2) for nl in N2_LISTS]

    tp = np.arange(52)[None, :]
    tau = np.arange(HOP)[:, None]
    l = HOP * tp + tau
    svs = [(P[bb] / cov[l]).astype(np.float32) for bb in range(b)]

    in_maps, shards = [], []
    for bb in range(b):
        for mm in range(m):
            fr_ = _frame(x_real[bb, :, mm])
            fi_ = _frame(x_imag[bb, :, mm])
            xfv = np.concatenate([-fi_, fr_, fi_], axis=1).astype(bfloat16)
            for h in range(2):
                in_maps.append({
                    "xf": xfv,
                    "fr_c": fr_c,
                    "gr_c": gr_c,
                    "smat": smats[h],
                    "mst": msts[h],
                    "svec": svs[bb],
                })
                shards.append((bb, mm, h))
    return in_maps, shards, P, cov


_NC_CACHE = {}


def kernel(x_real, x_imag, task_info, w_real, w_imag, b_real, b_imag):
    x_real = np.asarray(x_real)
    x_imag = np.asarray(x_imag)
    task_info = np.asarray(task_info)
    b, Lx, m = x_real.shape
    assert (b, Lx, m) == (2, L, 2)

    if "nc" not in _NC_CACHE:
        nc_ = build_program(debug=False)
        nc_.compile()
        _NC_CACHE["nc"] = nc_
    nc = _NC_CACHE["nc"]

    in_maps, shards, P, cov = make_in_maps(x_real, x_imag, task_info, w_real, w_imag)
    from concourse.bass_utils import run_bass_kernel_spmd
    res = run_bass_kernel_spmd(nc, in_maps, list(range(8))).results

    x = (x_real + 1j * x_imag).astype(np.complex64)
    out = x.copy()
    bias = complex(np.asarray(b_real)[0], np.asarray(b_imag)[0])
    bias_sig = np.zeros(L, np.complex64)
    bias_sig[np.arange(T) * HOP] = bias
    bias_sig /= cov
    for i, (bb, mm, h) in enumerate(shards):
        yvv = res[i]["yv"]          # [40, 104] = [tau, (yr(52) | yi(52))]
        yr = yvv[:, 0:52].T.ravel()[:L]
        yi = yvv[:, 52:104].T.ravel()[:L]
        out[bb, :, mm] += yr + 1j * yi
    for bb in range(b):
        for mm in range(m):
            out[bb, :, mm] += (P[bb] * bias_sig).astype(np.complex64)
    return out[:, 20:L - 20, :]



# revision 2
# speedup vs baseline: 1.0371x; 1.0371x over previous
"""Trainium2 Bass kernel for nn_EqStftPBC (STFT perturbation-based compensation).

v2: j-split sharding.  Core c handles n2 in {5c-20 .. 5c-16} for ALL four
(b, m) signals; host sums the 8 partial deltas.  Weights (M_j, G) are shared
across the 4 signals -> wide matmuls (N=408).  The core-specific base shift
5c-20 is folded into per-core STFT weights (Xs = roll(X, 5c-20) computed
directly); the residual shifts r=0..4 are identical across cores and done
with SBUF->SBUF partition-shifted DMAs -> single SPMD program.

Other folds: P^(1/3) scaling of the input frames (delta is cubic in x),
1/cov into the G weights, overlap-add edge factors into tail copies.
"""

import numpy as np
from ml_dtypes import bfloat16

import concourse.bass as bass
import concourse.bacc as bacc
import concourse.mybir as mybir
import concourse.tile as tile

F = 80
T = 51
TP = 52
HOP = 40
L = 2080
BM = 4            # (b, m) units, all on every core
NJ = 5            # n2 per core: n2 = 5*core - 20 + r, r in 0..NJ-1
CW = BM * TP      # 208: one (bm, t) plane slot per j
RW = 2 * CW       # 416: [r|i] slot per j
W1 = NJ * CW      # 1040: full plane width
FP32 = mybir.dt.float32
BF16 = mybir.dt.bfloat16
CPY = mybir.ActivationFunctionType.Copy


def _ap(t_ap, off, dims):
    """AP into a tile: keep the tile's partition dim, custom free dims."""
    return bass.AP(tensor=t_ap.tensor, offset=t_ap.offset + off,
                   ap=[t_ap.ap[0]] + dims)


def _app(t_ap, p0, np_, off, dims, pitch):
    """AP with partition sub-range [p0, p0+np_) and custom free dims."""
    return bass.AP(tensor=t_ap.tensor, offset=t_ap.offset + p0 * pitch + off,
                   ap=[[pitch, np_]] + dims)


def build_program(debug=False):
    nc = bacc.Bacc("TRN2", target_bir_lowering=False, debug=debug)

    xf = nc.dram_tensor("xf", [F, 3 * BM * T], BF16, kind="ExternalInput")
    fw = nc.dram_tensor("fw", [F, 4 * F], BF16, kind="ExternalInput")
    mw = nc.dram_tensor("mw", [F, NJ * 3 * F], BF16, kind="ExternalInput")
    gw = nc.dram_tensor("gw", [F, 3 * F], BF16, kind="ExternalInput")
    yv = nc.dram_tensor("yv", [HOP, 2 * CW], FP32, kind="ExternalOutput")

    MUL = mybir.AluOpType.mult
    ADD = mybir.AluOpType.add
    SUB = mybir.AluOpType.subtract

    with tile.TileContext(nc) as tc:
        with (
            tc.tile_pool(name="const", bufs=1) as cpool,
            tc.tile_pool(name="work", bufs=1) as wpool,
            tc.tile_pool(name="ps_s", bufs=1, space="PSUM") as ps_s,
            tc.tile_pool(name="ps_u", bufs=3, space="PSUM") as ps_u,
        ):
            # ---- input DMAs ----
            xfs = wpool.tile([F, 3 * BM * T], BF16, tag="xfs")
            nc.sync.dma_start(xfs[:, :], xf[:, :])
            fws = cpool.tile([F, 4 * F], BF16, tag="fws")
            nc.sync.dma_start(fws[:, :], fw[:, :])
            mws = cpool.tile([F, NJ * 3 * F], BF16, tag="mws")
            HM = NJ * 3 * F // 2
            nc.gpsimd.dma_start(mws[:, 0:HM], mw[:, 0:HM])
            nc.gpsimd.dma_start(mws[:, HM:2 * HM], mw[:, HM:2 * HM])
            gws = cpool.tile([F, 3 * F], BF16, tag="gws")
            nc.gpsimd.dma_start(gws[:, :], gw[:, :])

            # ---- STFT: X0 = W @ frames, Xs = W_rolled @ frames ----
            BT = BM * T  # 204
            X0p = ps_s.tile([F, 2 * BT], FP32, tag="X0p")
            Xsp = ps_s.tile([F, 2 * BT], FP32, tag="Xsp")
            nc.tensor.matmul(X0p[:, :], fws[:, 0:F], xfs[:, BT:3 * BT],
                             start=True, stop=False)
            nc.tensor.matmul(X0p[:, :], fws[:, F:2 * F], xfs[:, 0:2 * BT],
                             start=False, stop=True)
            nc.tensor.matmul(Xsp[:, :], fws[:, 2 * F:3 * F], xfs[:, BT:3 * BT],
                             start=True, stop=False)
            nc.tensor.matmul(Xsp[:, :], fws[:, 3 * F:4 * F], xfs[:, 0:2 * BT],
                             start=False, stop=True)

            X0 = wpool.tile([F, RW], BF16, tag="X0")
            Rall = wpool.tile([F, NJ * RW], BF16, tag="Rall")
            psv = [[BT, 2], [T, BM], [1, T]]
            sbv = [[CW, 2], [TP, BM], [1, T]]
            nc.scalar.activation(_ap(X0[:, :], 0, sbv), _ap(X0p[:, :], 0, psv), CPY)
            nc.scalar.activation(_ap(Rall[:, :], 0, sbv), _ap(Xsp[:, :], 0, psv), CPY)

            # ---- roll DMAs: Rall slot r = roll(Xs, r) over partitions ----
            for r in range(1, NJ):
                eng = nc.sync if r % 2 else nc.gpsimd
                eng.dma_start(Rall[r:F, r * RW:(r + 1) * RW],
                              Rall[0:F - r, 0:RW])
                eng.dma_start(Rall[0:r, r * RW:(r + 1) * RW],
                              Rall[F - r:F, 0:RW])

            # ---- C stage ----
            sA = wpool.tile([F, W1], BF16, tag="sA")
            sB = wpool.tile([F, W1], BF16, tag="sB")
            sC = wpool.tile([F, W1], BF16, tag="sC")
            sD = wpool.tile([F, W1], BF16, tag="sD")
            Cp = wpool.tile([F, 2 * W1], BF16, tag="Cp")
            Call = wpool.tile([F, 2 * W1], BF16, tag="Call")

            TTv = nc.vector.tensor_tensor
            TTg = nc.gpsimd.tensor_tensor

            def c_group(j0, nj):
                o = j0 * CW
                u = nj * BM
                x0r = X0[:, None, 0:CW].to_broadcast([F, nj, CW])
                x0i = X0[:, None, CW:RW].to_broadcast([F, nj, CW])
                rr = _ap(Rall[:, :], j0 * RW, [[RW, nj], [1, CW]])
                ri = _ap(Rall[:, :], j0 * RW + CW, [[RW, nj], [1, CW]])
                vA = _ap(sA[:, :], o, [[CW, nj], [1, CW]])
                vB = _ap(sB[:, :], o, [[CW, nj], [1, CW]])
                vC = _ap(sC[:, :], o, [[CW, nj], [1, CW]])
                vD = _ap(sD[:, :], o, [[CW, nj], [1, CW]])
                TTv(vA, x0r, rr, MUL)
                TTg(vB, x0i, ri, MUL)
                TTv(vC, x0i, rr, MUL)
                TTg(vD, x0r, ri, MUL)
                # combines: write C_pre at col t+1 within each 52-slot
                s_in = [[TP, u], [1, T]]
                TTv(_ap(Cp[:, :], o + 1, s_in),
                    _ap(sA[:, :], o, s_in), _ap(sB[:, :], o, s_in), ADD)
                TTg(_ap(Cp[:, :], W1 + o + 1, s_in),
                    _ap(sC[:, :], o, s_in), _ap(sD[:, :], o, s_in), SUB)
                # wrap: col0 <- col51 (both planes, one op)
                nc.scalar.activation(
                    _ap(Cp[:, :], o, [[W1, 2], [TP, u]]),
                    _ap(Cp[:, :], o + T, [[W1, 2], [TP, u]]), CPY)
                # roll-add: Call[t] = Cp[t] + Cp[t-1]
                TTv(_ap(Call[:, :], o, s_in),
                    _ap(Cp[:, :], o + 1, s_in), _ap(Cp[:, :], o, s_in), ADD)
                TTg(_ap(Call[:, :], W1 + o, s_in),
                    _ap(Cp[:, :], W1 + o + 1, s_in), _ap(Cp[:, :], W1 + o, s_in),
                    ADD)

            # ---- per-j stages ----
            Usb = wpool.tile([F, NJ * RW], BF16, tag="Usb")
            Vall = wpool.tile([F, 2 * W1], BF16, tag="Vall")
            Yp = ps_s.tile([F, 2 * BT], FP32, tag="Yp")
            Ups = [None] * NJ

            def u_mm(j):
                Up = ps_u.tile([F, 2 * BT], FP32, tag="Up")
                Ups[j] = Up
                o = j * CW
                rhs_both = _ap(Call[:, :], o, [[W1, 2], [TP, BM], [1, T]])
                rhs_i = _ap(Call[:, :], W1 + o, [[TP, BM], [1, T]])
                rhs_r = _ap(Call[:, :], o, [[TP, BM], [1, T]])
                mo = j * 3 * F
                nc.tensor.matmul(Up[:, :], mws[:, mo:mo + F], rhs_both,
                                 start=True, stop=False)
                nc.tensor.matmul(Up[:, 0:BT], mws[:, mo + F:mo + 2 * F], rhs_i,
                                 start=False, stop=False)
                nc.tensor.matmul(Up[:, BT:2 * BT], mws[:, mo + 2 * F:mo + 3 * F],
                                 rhs_r, start=False, stop=True)

            def u_evict(j):
                nc.scalar.activation(_ap(Usb[:, :], j * RW, sbv),
                                     _ap(Ups[j][:, :], 0, psv), CPY)

            def v_tt(j):
                o = j * CW
                ur = Usb[:, j * RW:j * RW + CW]
                ui = Usb[:, j * RW + CW:(j + 1) * RW]
                rr = Rall[:, j * RW:j * RW + CW]
                ri = Rall[:, j * RW + CW:(j + 1) * RW]
                ta = sA[:, o:o + CW]
                tb = sB[:, o:o + CW]
                tc_ = sC[:, o:o + CW]
                td = sD[:, o:o + CW]
                TTv(ta, ur, rr, MUL)
                TTg(tb, ui, ri, MUL)
                TTv(td, ui, rr, MUL)
                TTg(tc_, ur, ri, MUL)
                TTv(Vall[:, o:o + CW], ta, tb, SUB)
                TTg(Vall[:, W1 + o:W1 + o + CW], tc_, td, ADD)

            def d_mm(j):
                o = j * CW
                rhs_both = _ap(Vall[:, :], o, [[W1, 2], [TP, BM], [1, T]])
                rhs_i = _ap(Vall[:, :], W1 + o, [[TP, BM], [1, T]])
                rhs_r = _ap(Vall[:, :], o, [[TP, BM], [1, T]])
                nc.tensor.matmul(Yp[:, :], gws[:, 0:F], rhs_both,
                                 start=(j == 0), stop=False)
                nc.tensor.matmul(Yp[:, 0:BT], gws[:, 2 * F:3 * F], rhs_i,
                                 start=False, stop=False)
                nc.tensor.matmul(Yp[:, BT:2 * BT], gws[:, F:2 * F], rhs_r,
                                 start=False, stop=(j == NJ - 1))

            # ---- software-pipelined emission ----
            c_group(0, 1)
            u_mm(0)
            c_group(1, NJ - 1)
            u_mm(1)
            u_evict(0)
            v_tt(0)
            u_mm(2)
            d_mm(0)
            u_evict(1)
            v_tt(1)
            u_mm(3)
            d_mm(1)
            u_evict(2)
            v_tt(2)
            u_mm(4)
            d_mm(2)
            u_evict(3)
            v_tt(3)
            d_mm(3)
            u_evict(4)
            v_tt(4)
            d_mm(4)

            # ---- tail: overlap-add (cov folded into gw, edges fixed here) ----
            Dsb = wpool.tile([F, 2 * BT], FP32, tag="Dsb")
            nc.scalar.activation(Dsb[:, :], Yp[:, :], CPY)
            S2 = wpool.tile([HOP, 2 * BT], FP32, tag="S2")
            nc.sync.dma_start(S2[:, :], Dsb[HOP:F, :])
            Ysb = wpool.tile([HOP, 2 * CW], FP32, tag="Ysb")
            for c2 in range(2):
                TTv(_ap(Ysb[:, :], c2 * CW + 1, [[TP, BM], [1, T - 1]]),
                    _ap(Dsb[0:HOP, :], c2 * BT + 1, [[T, BM], [1, T - 1]]),
                    _ap(S2[:, :], c2 * BT, [[T, BM], [1, T - 1]]), ADD)
            # edges carry the x2 (cov=1 there, gw carries 1/2)
            nc.scalar.activation(
                _ap(Ysb[:, :], 0, [[CW, 2], [TP, BM]]),
                _ap(Dsb[0:HOP, :], 0, [[BT, 2], [T, BM]]), CPY, scale=2.0)
            nc.scalar.activation(
                _ap(Ysb[:, :], T, [[CW, 2], [TP, BM]]),
                _ap(S2[:, :], T - 1, [[BT, 2], [T, BM]]), CPY, scale=2.0)
            nc.sync.dma_start(yv[:, :], Ysb[:, :])
    return nc


# ---------------- host side ----------------

def _dft_consts():
    j = np.arange(F)
    W = np.exp(-2j * np.pi * np.outer(j, j) / F)
    G = np.exp(+2j * np.pi * np.outer(j, j) / F) / F
    return W, G


def _frame(sig):
    idx = np.arange(T)[None, :] * HOP + np.arange(F)[:, None]   # [g, t]
    return sig[idx].astype(np.float32)


def _m_mats(w2, n2):
    g = np.arange(F)[:, None]
    f = np.arange(F)[None, :]
    n1 = ((f - g + 20) % F) - 20
    valid = (n1 >= -20) & (n1 <= 19)
    n1c = np.clip(n1 + 20, 0, 39)
    col = w2[:, n2 + 20]
    Mr = np.where(valid, col.real[n1c], 0.0).astype(np.float32)
    Mi = np.where(valid, col.imag[n1c], 0.0).astype(np.float32)
    return Mr, Mi


def make_in_maps(x_real, x_imag, task_info, w_real, w_imag):
    W, G = _dft_consts()
    b, _, m = x_real.shape
    P = np.power(10.0, task_info[:, 0] / 10.0) / m
    w2 = (np.asarray(w_real) + 1j * np.asarray(w_imag)).reshape(40, 40)

    # frames scaled by P^(1/3) per bm (delta is cubic in x)
    frs, fis = [], []
    for bb in range(b):
        s = float(P[bb]) ** (1.0 / 3.0)
        for mm in range(m):
            frs.append(_frame(x_real[bb, :, mm]) * s)
            fis.append(_frame(x_imag[bb, :, mm]) * s)
    fr = np.stack(frs, 1)   # [g, bm, t]
    fi = np.stack(fis, 1)
    xfv = np.concatenate([(-fi).reshape(F, -1), fr.reshape(F, -1),
                          fi.reshape(F, -1)], axis=1).astype(bfloat16)

    # G weights with 1/cov=1/2 folded in
    gwv = np.concatenate([G.real * 0.5, G.imag * 0.5, -G.imag * 0.5],
                         axis=1).astype(bfloat16)

    in_maps, shards = [], []
    for ci in range(8):
        sc = 5 * ci - 20
        Ws = np.roll(W, sc, axis=0).T
        fwv = np.concatenate([W.real, W.imag, Ws.real, Ws.imag],
                             axis=1).astype(bfloat16)
        mparts = []
        for r in range(NJ):
            Mr, Mi = _m_mats(w2, sc + r)
            mparts += [Mr, -Mi, Mi]
        mwv = np.concatenate(mparts, axis=1).astype(bfloat16)
        in_maps.append({"xf": xfv, "fw": fwv, "mw": mwv, "gw": gwv})
        shards.append(ci)

    cov = np.zeros(L)
    idx = (np.arange(T)[:, None] * HOP + np.arange(F)[None, :]).reshape(-1)
    np.add.at(cov, idx, 1.0)
    cov = np.where(cov > 0, cov, 1.0)
    return in_maps, shards, P, cov


_NC_CACHE = {}


def kernel(x_real, x_imag, task_info, w_real, w_imag, b_real, b_imag):
    x_real = np.asarray(x_real)
    x_imag = np.asarray(x_imag)
    task_info = np.asarray(task_info)
    b, Lx, m = x_real.shape
    assert (b, Lx, m) == (2, L, 2)

    if "nc" not in _NC_CACHE:
        nc_ = build_program(debug=False)
        nc_.compile()
        _NC_CACHE["nc"] = nc_
    nc = _NC_CACHE["nc"]

    in_maps, shards, P, cov = make_in_maps(x_real, x_imag, task_info,
                                           w_real, w_imag)
    from concourse.bass_utils import run_bass_kernel_spmd
    res = run_bass_kernel_spmd(nc, in_maps, list(range(8))).results

    Ysum = np.zeros((HOP, 2 * CW), np.float64)
    for i in range(8):
        Ysum += np.asarray(res[i]["yv"], np.float64)
    Y = Ysum.reshape(HOP, 2, BM, TP)

    x = (x_real + 1j * x_imag).astype(np.complex64)
    out = x.copy()
    bias = complex(np.asarray(b_real)[0], np.asarray(b_imag)[0])
    bias_sig = np.zeros(L, np.complex64)
    bias_sig[np.arange(T) * HOP] = bias
    bias_sig /= cov
    for u in range(BM):
        bb, mm = divmod(u, m)
        yr = Y[:, 0, u].T.ravel()[:L]
        yi = Y[:, 1, u].T.ravel()[:L]
        out[bb, :, mm] += (yr + 1j * yi).astype(np.complex64)
        out[bb, :, mm] += (P[bb] * bias_sig).astype(np.complex64)
    return out[:, 20:L - 20, :]


# revision 8
# speedup vs baseline: 1.3823x; 1.3328x over previous
"""Trainium2 Bass kernel for nn_EqStftPBC (STFT perturbation-based compensation).

v3: j-split sharding (core c: n2 in {5c-20..5c-16}, all 4 (b,m) signals),
host sums the 8 partial deltas.

- Per-core base shift folded into STFT weights (Xs); residual rolls r=1..4
  as permutation matmuls on the otherwise-idle PE.
- 51-dense plane-major layouts: one DVE op computes two real-product planes,
  evictions are flat copies.
- P^(1/3) folded into frames (delta is cubic in x), 1/cov into G weights,
  overlap-add folded into split G weights (Ga/Gb -> two 40-row PSUM banks).
"""

import numpy as np
from ml_dtypes import bfloat16

import concourse.bass as bass
import concourse.bacc as bacc
import concourse.mybir as mybir
import concourse.tile as tile

F = 80
T = 51
TP = 52
HOP = 40
L = 2080
BM = 4            # (b, m) units, all on every core
NJ = 5            # n2 per core: n2 = 5*core - 20 + r
CD = BM * T       # 204: dense (bm, t) slot per (plane, j)
WD = NJ * CD      # 1020: one plane across all j
FP32 = mybir.dt.float32
BF16 = mybir.dt.bfloat16
CPY = mybir.ActivationFunctionType.Copy


def _ap(t_ap, off, dims):
    return bass.AP(tensor=t_ap.tensor, offset=t_ap.offset + off,
                   ap=[t_ap.ap[0]] + dims)


def build_program(debug=False):
    nc = bacc.Bacc("TRN2", target_bir_lowering=False, debug=debug)

    xf = nc.dram_tensor("xf", [F, 3 * CD], BF16, kind="ExternalInput")
    fw = nc.dram_tensor("fw", [F, 4 * F], BF16, kind="ExternalInput")
    pw = nc.dram_tensor("pw", [F, 4 * F], BF16, kind="ExternalInput")
    mw = nc.dram_tensor("mw", [F, NJ * 3 * F], BF16, kind="ExternalInput")
    gw = nc.dram_tensor("gw", [F, 6 * HOP], BF16, kind="ExternalInput")
    yv = nc.dram_tensor("yv", [HOP, 2 * BM * TP], FP32, kind="ExternalOutput")

    MUL = mybir.AluOpType.mult
    ADD = mybir.AluOpType.add
    SUB = mybir.AluOpType.subtract

    with tile.TileContext(nc) as tc:
        with (
            tc.tile_pool(name="const", bufs=1) as cpool,
            tc.tile_pool(name="work", bufs=1) as wpool,
            tc.tile_pool(name="ps_s", bufs=1, space="PSUM") as ps_s,
            tc.tile_pool(name="ps_u", bufs=3, space="PSUM") as ps_u,
        ):
            # ---- input DMAs on 4 queues ----
            xfs = wpool.tile([F, 3 * CD], BF16, tag="xfs")
            nc.sync.dma_start(xfs[:, :], xf[:, :])
            fws = cpool.tile([F, 4 * F], BF16, tag="fws")
            nc.scalar.dma_start(fws[:, :], fw[:, :])
            pws = cpool.tile([F, 4 * F], BF16, tag="pws")
            nc.scalar.dma_start(pws[:, :], pw[:, :])
            mws = cpool.tile([F, NJ * 3 * F], BF16, tag="mws")
            HM = NJ * 3 * F // 2
            nc.gpsimd.dma_start(mws[:, 0:HM], mw[:, 0:HM])
            nc.sync.dma_start(mws[:, HM:2 * HM], mw[:, HM:2 * HM])
            gws = cpool.tile([F, 6 * HOP], BF16, tag="gws")
            nc.gpsimd.dma_start(gws[:, :], gw[:, :])

            # ---- STFT (Xs first: slot0 gates the R matmuls) ----
            Xsp = ps_s.tile([F, 2 * CD], FP32, tag="Xsp")
            X0p = ps_s.tile([F, 2 * CD], FP32, tag="X0p")
            nc.tensor.matmul(Xsp[:, :], fws[:, 2 * F:3 * F], xfs[:, CD:3 * CD],
                             start=True, stop=False)
            nc.tensor.matmul(Xsp[:, :], fws[:, 3 * F:4 * F], xfs[:, 0:2 * CD],
                             start=False, stop=True)
            nc.tensor.matmul(X0p[:, :], fws[:, 0:F], xfs[:, CD:3 * CD],
                             start=True, stop=False)
            nc.tensor.matmul(X0p[:, :], fws[:, F:2 * F], xfs[:, 0:2 * CD],
                             start=False, stop=True)

            # Rall: plane-major [Rr(5j) | Ri(5j)], slot j = roll(Xs, j)
            Rall = wpool.tile([F, 2 * WD], BF16, tag="Rall")
            nc.scalar.activation(_ap(Rall[:, :], 0, [[WD, 2], [1, CD]]),
                                 Xsp[:, :], CPY)

            # X0T: [X0r x5 | X0i x5] tiled across j slots
            X0T = wpool.tile([F, 2 * WD], BF16, tag="X0T")
            nc.scalar.activation(_ap(X0T[:, :], 0, [[WD, 2], [1, CD]]),
                                 X0p[:, :], CPY)
            for pl in range(2):
                nc.vector.tensor_copy(
                    _ap(X0T[:, :], pl * WD + CD, [[1, 4 * CD]]),
                    X0T[:, None, pl * WD:pl * WD + CD].to_broadcast(
                        [F, 4, CD]))

            # ---- residual rolls via permutation matmuls ----
            for r in range(1, NJ):
                Rp = ps_u.tile([F, 2 * CD], FP32, tag="Up")
                rhs = _ap(Rall[:, :], 0, [[WD, 2], [1, CD]])
                nc.tensor.matmul(Rp[:, :], pws[:, (r - 1) * F:r * F], rhs,
                                 start=True, stop=True)
                dst = _ap(Rall[:, :], r * CD, [[WD, 2], [1, CD]])
                src = _ap(Rp[:, :], 0, [[CD, 2], [1, CD]])
                nc.scalar.activation(dst, src, CPY)

            # ---- C stage (grouped) ----
            CS = wpool.tile([F, 4 * WD], BF16, tag="CS")   # [sA5|sC5|sB5|sD5]
            Cp = wpool.tile([F, 2 * WD], BF16, tag="Cp")
            Call = wpool.tile([F, 2 * WD], BF16, tag="Call")
            TTv = nc.vector.tensor_tensor
            TTg = nc.gpsimd.tensor_tensor

            def c_group(j0, nj):
                o = j0 * CD
                n = nj * CD
                u = nj * BM
                # [sA|sC] = [X0r|X0i] (x) Rr ; [sB|sD] = [X0i|X0r] (x) Ri
                TTv(_ap(CS[:, :], o, [[WD, 2], [1, n]]),
                    _ap(X0T[:, :], o, [[WD, 2], [1, n]]),
                    _ap(Rall[:, :], o, [[0, 2], [1, n]]), MUL)
                TTv(_ap(CS[:, :], 2 * WD + o, [[WD, 2], [1, n]]),
                    _ap(X0T[:, :], WD + o, [[-WD, 2], [1, n]]),
                    _ap(Rall[:, :], WD + o, [[0, 2], [1, n]]), MUL)
                # Crp = sA+sB ; Cip = sC-sD   (flat, into Cp planes)
                TTv(_ap(Cp[:, :], o, [[1, n]]),
                    _ap(CS[:, :], o, [[1, n]]),
                    _ap(CS[:, :], 2 * WD + o, [[1, n]]), ADD)
                TTv(_ap(Cp[:, :], WD + o, [[1, n]]),
                    _ap(CS[:, :], WD + o, [[1, n]]),
                    _ap(CS[:, :], 3 * WD + o, [[1, n]]), SUB)
                # roll-add over t within each (j,bm) block of 51
                TTv(_ap(Call[:, :], o + 1, [[T, u], [1, T - 1]]),
                    _ap(Cp[:, :], o + 1, [[T, u], [1, T - 1]]),
                    _ap(Cp[:, :], o, [[T, u], [1, T - 1]]), ADD)
                TTg(_ap(Call[:, :], WD + o + 1, [[T, u], [1, T - 1]]),
                    _ap(Cp[:, :], WD + o + 1, [[T, u], [1, T - 1]]),
                    _ap(Cp[:, :], WD + o, [[T, u], [1, T - 1]]), ADD)
                TTv(_ap(Call[:, :], o, [[T, u]]),
                    _ap(Cp[:, :], o, [[T, u]]),
                    _ap(Cp[:, :], o + T - 1, [[T, u]]), ADD)
                TTg(_ap(Call[:, :], WD + o, [[T, u]]),
                    _ap(Cp[:, :], WD + o, [[T, u]]),
                    _ap(Cp[:, :], WD + o + T - 1, [[T, u]]), ADD)

            # ---- per-j stages ----
            Usb = wpool.tile([F, NJ * 2 * CD], BF16, tag="Usb")
            VS = wpool.tile([F, 8 * CD], BF16, tag="VS")
            Vall = wpool.tile([F, 2 * WD], BF16, tag="Vall")
            Ya = ps_s.tile([HOP, 2 * CD], FP32, tag="Ya")
            Yb = ps_s.tile([HOP, 2 * CD], FP32, tag="Yb")
            Ups = [None] * NJ

            def u_mm(j):
                Up = ps_u.tile([F, 2 * CD], FP32, tag="Up")
                Ups[j] = Up
                rhs2 = _ap(Call[:, :], j * CD, [[WD, 2], [1, CD]])
                rhs_i = _ap(Call[:, :], WD + j * CD, [[1, CD]])
                rhs_r = _ap(Call[:, :], j * CD, [[1, CD]])
                mo = j * 3 * F
                nc.tensor.matmul(Up[:, :], mws[:, mo:mo + F], rhs2,
                                 start=True, stop=False)
                nc.tensor.matmul(Up[:, 0:CD], mws[:, mo + F:mo + 2 * F], rhs_i,
                                 start=False, stop=False)
                nc.tensor.matmul(Up[:, CD:2 * CD], mws[:, mo + 2 * F:mo + 3 * F],
                                 rhs_r, start=False, stop=True)

            def u_evict(j):
                if j % 2:
                    nc.vector.tensor_copy(Usb[:, j * 2 * CD:(j + 1) * 2 * CD],
                                          Ups[j][:, :])
                else:
                    nc.scalar.activation(Usb[:, j * 2 * CD:(j + 1) * 2 * CD],
                                         Ups[j][:, :], CPY)

            def v_tt(j):
                p = (j % 2) * 4 * CD
                uu = Usb[:, j * 2 * CD:(j + 1) * 2 * CD]
                # [tA|tB] = [Ur|Ui] (x) [Rr|Ri] ; [tC|tD] = [Ur|Ui] (x) [Ri|Rr]
                TTv(_ap(VS[:, :], p, [[1, 2 * CD]]), uu,
                    _ap(Rall[:, :], j * CD, [[WD, 2], [1, CD]]), MUL)
                TTv(_ap(VS[:, :], p + 2 * CD, [[1, 2 * CD]]), uu,
                    _ap(Rall[:, :], WD + j * CD, [[-WD, 2], [1, CD]]), MUL)
                # Vr = tA - tB ; Vi = tC + tD
                TTv(_ap(Vall[:, :], j * 2 * CD, [[1, CD]]),
                    _ap(VS[:, :], p, [[1, CD]]),
                    _ap(VS[:, :], p + CD, [[1, CD]]), SUB)
                TTg(_ap(Vall[:, :], j * 2 * CD + CD, [[1, CD]]),
                    _ap(VS[:, :], p + 2 * CD, [[1, CD]]),
                    _ap(VS[:, :], p + 3 * CD, [[1, CD]]), ADD)

            def d_mm(j):
                rhs2 = _ap(Vall[:, :], j * 2 * CD, [[CD, 2], [1, CD]])
                rhs_i = _ap(Vall[:, :], j * 2 * CD + CD, [[1, CD]])
                rhs_r = _ap(Vall[:, :], j * 2 * CD, [[1, CD]])
                st = (j == 0)
                sp = (j == NJ - 1)
                nc.tensor.matmul(Ya[:, :], gws[:, 0:HOP], rhs2,
                                 start=st, stop=False)
                nc.tensor.matmul(Yb[:, :], gws[:, HOP:2 * HOP], rhs2,
                                 start=st, stop=False)
                nc.tensor.matmul(Ya[:, 0:CD], gws[:, 2 * HOP:3 * HOP], rhs_i,
                                 start=False, stop=False)
                nc.tensor.matmul(Yb[:, 0:CD], gws[:, 3 * HOP:4 * HOP], rhs_i,
                                 start=False, stop=False)
                nc.tensor.matmul(Ya[:, CD:2 * CD], gws[:, 4 * HOP:5 * HOP],
                                 rhs_r, start=False, stop=sp)
                nc.tensor.matmul(Yb[:, CD:2 * CD], gws[:, 5 * HOP:6 * HOP],
                                 rhs_r, start=False, stop=sp)

            # ---- software-pipelined emission ----
            c_group(0, 2)
            u_mm(0)
            u_mm(1)
            c_group(2, 3)
            u_evict(0)
            v_tt(0)
            u_mm(2)
            d_mm(0)
            u_evict(1)
            v_tt(1)
            u_mm(3)
            d_mm(1)
            u_evict(2)
            v_tt(2)
            u_mm(4)
            d_mm(2)
            u_evict(3)
            v_tt(3)
            d_mm(3)
            u_evict(4)
            v_tt(4)
            d_mm(4)

            # ---- tail: Y[tp] = Ya[t=tp] + Yb[t=tp-1], edges x2 ----
            Ysb = wpool.tile([HOP, 2 * BM * TP], FP32, tag="Ysb")
            Ybs = wpool.tile([HOP, 2 * CD], FP32, tag="Ybs")
            nc.scalar.activation(Ybs[:, :], Yb[:, :], CPY)
            CW = BM * TP
            for c2 in range(2):
                TTv(_ap(Ysb[:, :], c2 * CW + 1, [[TP, BM], [1, T - 1]]),
                    _ap(Ya[:, :], c2 * CD + 1, [[T, BM], [1, T - 1]]),
                    _ap(Ybs[:, :], c2 * CD, [[T, BM], [1, T - 1]]), ADD)
            nc.scalar.activation(
                _ap(Ysb[:, :], 0, [[CW, 2], [TP, BM]]),
                _ap(Ya[:, :], 0, [[CD, 2], [T, BM]]), CPY, scale=2.0)
            nc.scalar.activation(
                _ap(Ysb[:, :], T, [[CW, 2], [TP, BM]]),
                _ap(Ybs[:, :], T - 1, [[CD, 2], [T, BM]]), CPY, scale=2.0)
            nc.sync.dma_start(yv[:, :], Ysb[:, :])
    return nc


# ---------------- host side ----------------

def _dft_consts():
    j = np.arange(F)
    W = np.exp(-2j * np.pi * np.outer(j, j) / F)
    G = np.exp(+2j * np.pi * np.outer(j, j) / F) / F
    return W, G


def _frame(sig):
    idx = np.arange(T)[None, :] * HOP + np.arange(F)[:, None]   # [g, t]
    return sig[idx].astype(np.float32)


def _m_mats(w2, n2):
    g = np.arange(F)[:, None]
    f = np.arange(F)[None, :]
    n1 = ((f - g + 20) % F) - 20
    valid = (n1 >= -20) & (n1 <= 19)
    n1c = np.clip(n1 + 20, 0, 39)
    col = w2[:, n2 + 20]
    Mr = np.where(valid, col.real[n1c], 0.0).astype(np.float32)
    Mi = np.where(valid, col.imag[n1c], 0.0).astype(np.float32)
    return Mr, Mi


def make_in_maps(x_real, x_imag, task_info, w_real, w_imag):
    W, G = _dft_consts()
    b, _, m = x_real.shape
    P = np.power(10.0, task_info[:, 0] / 10.0) / m
    w2 = (np.asarray(w_real) + 1j * np.asarray(w_imag)).reshape(40, 40)

    frs, fis = [], []
    for bb in range(b):
        s = float(P[bb]) ** (1.0 / 3.0)
        for mm in range(m):
            frs.append(_frame(x_real[bb, :, mm]) * s)
            fis.append(_frame(x_imag[bb, :, mm]) * s)
    fr = np.stack(frs, 1)
    fi = np.stack(fis, 1)
    xfv = np.concatenate([(-fi).reshape(F, -1), fr.reshape(F, -1),
                          fi.reshape(F, -1)], axis=1).astype(bfloat16)

    # G folded: 1/cov=1/2, rows split [0:40)/[40:80) for fused overlap-add
    Gh = G * 0.5
    gwv = np.concatenate([Gh.real[0:HOP].T, Gh.real[HOP:F].T,
                          -Gh.imag[0:HOP].T, -Gh.imag[HOP:F].T,
                          Gh.imag[0:HOP].T, Gh.imag[HOP:F].T],
                         axis=1).astype(bfloat16)

    # permutation matrices for rolls r=1..4 (lhsT[g, f] = 1 iff g=(f-r)%80)
    pparts = []
    g = np.arange(F)
    for r in range(1, NJ):
        Pm = np.zeros((F, F), np.float32)
        Pm[(g - r) % F, g] = 1.0
        pparts.append(Pm)
    pwv = np.concatenate(pparts, axis=1).astype(bfloat16)

    in_maps, shards = [], []
    for ci in range(8):
        sc = 5 * ci - 20
        Ws = np.roll(W, sc, axis=0).T
        fwv = np.concatenate([W.real, W.imag, Ws.real, Ws.imag],
                             axis=1).astype(bfloat16)
        mparts = []
        for r in range(NJ):
            Mr, Mi = _m_mats(w2, sc + r)
            mparts += [Mr, -Mi, Mi]
        mwv = np.concatenate(mparts, axis=1).astype(bfloat16)
        in_maps.append({"xf": xfv, "fw": fwv, "pw": pwv, "mw": mwv,
                        "gw": gwv})
        shards.append(ci)

    cov = np.zeros(L)
    idx = (np.arange(T)[:, None] * HOP + np.arange(F)[None, :]).reshape(-1)
    np.add.at(cov, idx, 1.0)
    cov = np.where(cov > 0, cov, 1.0)
    return in_maps, shards, P, cov


_NC_CACHE = {}


def kernel(x_real, x_imag, task_info, w_real, w_imag, b_real, b_imag):
    x_real = np.asarray(x_real)
    x_imag = np.asarray(x_imag)
    task_info = np.asarray(task_info)
    b, Lx, m = x_real.shape
    assert (b, Lx, m) == (2, L, 2)

    if "nc" not in _NC_CACHE:
        nc_ = build_program(debug=False)
        nc_.compile()
        _NC_CACHE["nc"] = nc_
    nc = _NC_CACHE["nc"]

    in_maps, shards, P, cov = make_in_maps(x_real, x_imag, task_info,
                                           w_real, w_imag)
    from concourse.bass_utils import run_bass_kernel_spmd
    res = run_bass_kernel_spmd(nc, in_maps, list(range(8))).results

    CW = BM * TP
    Ysum = np.zeros((HOP, 2 * CW), np.float64)
    for i in range(8):
        Ysum += np.asarray(res[i]["yv"], np.float64)
    Y = Ysum.reshape(HOP, 2, BM, TP)

    x = (x_real + 1j * x_imag).astype(np.complex64)
    out = x.copy()
    bias = complex(np.asarray(b_real)[0], np.asarray(b_imag)[0])
    bias_sig = np.zeros(L, np.complex64)
    bias_sig[np.arange(T) * HOP] = bias
    bias_sig /= cov
    for u in range(BM):
        bb, mm = divmod(u, m)
        yr = Y[:, 0, u].T.ravel()[:L]
        yi = Y[:, 1, u].T.ravel()[:L]
        out[bb, :, mm] += (yr + 1j * yi).astype(np.complex64)
        out[bb, :, mm] += (P[bb] * bias_sig).astype(np.complex64)
    return out[:, 20:L - 20, :]
